# revision 2
# baseline (speedup 1.0000x reference)
"""Low-rank linear kernel for Trainium2 (8 NeuronCores, data-parallel).

Computes out = input @ (A @ B).T with A:[4096,16], B:[16,4096],
input:[4,4096,4096] — via the rank-16 factorization:
    t.T = B @ x.T   (contract 4096)
    out = t @ A.T   (contract 16)
Input rows are sharded 8 ways across cores; A,B replicated.
"""
import numpy as np
import concourse.bass as bass
from concourse import bacc
import concourse.mybir as mybir
import concourse.tile as tile
from concourse.bass_utils import run_bass_kernel_spmd
from concourse.masks import make_identity

F32 = mybir.dt.float32

N_CORES = 8
ROWS_TOTAL = 4 * 4096
R = ROWS_TOTAL // N_CORES  # 2048 rows per core
D = 4096
RANK = 16
RT = 128                   # row tile
N_RT = R // RT             # 16 row tiles per core
KC = D // 128              # 32 contraction chunks
TG = 4                     # transpose group: chunks per PSUM bank / ACT copy
NJ = D // 512              # 8 output column chunks

_CACHE = {}


def _build():
    nc = bacc.Bacc("TRN2", debug=False, num_devices=N_CORES)
    x = nc.dram_tensor("x", [R, D], F32, kind="ExternalInput")
    a = nc.dram_tensor("a", [D, RANK], F32, kind="ExternalInput")
    b = nc.dram_tensor("b", [RANK, D], F32, kind="ExternalInput")
    y = nc.dram_tensor("y", [R, D], F32, kind="ExternalOutput")

    with tile.TileContext(nc) as tc:
        with (
            tc.tile_pool(name="const", bufs=1) as const,
            tc.tile_pool(name="pre_ps", bufs=1, space="PSUM") as pre_ps,
            tc.tile_pool(name="xin", bufs=3) as xin,
            tc.tile_pool(name="xt_ps", bufs=2, space="PSUM") as xt_ps_pool,
            tc.tile_pool(name="xt_sb", bufs=3) as xt_sb_pool,
            tc.tile_pool(name="tt_ps", bufs=2, space="PSUM") as tt_ps_pool,
            tc.tile_pool(name="tt_sb", bufs=2) as tt_sb_pool,
            tc.tile_pool(name="o_ps", bufs=3, space="PSUM") as o_ps_pool,
            tc.tile_pool(name="o_sb", bufs=2) as o_sb_pool,
        ):
            # ---- constants / preprocessing (once per core) ----
            ident = const.tile([128, 128], F32)
            make_identity(nc, ident[:])

            # B natural [16, D]
            b_nat = const.tile([RANK, D], F32)
            nc.sync.dma_start(b_nat[:], b[:])
            # BT[128, 16*KC]: chunk c at cols 16c:16c+16 equals B[:,128c:128c+128].T
            bt = const.tile([128, RANK * KC], F32)
            for c in range(KC):
                ps = pre_ps.tile([128, RANK], F32)
                nc.tensor.transpose(
                    ps[:], b_nat[:, 128 * c:128 * (c + 1)], ident[:RANK, :RANK]
                )
                nc.scalar.copy(bt[:, RANK * c:RANK * (c + 1)], ps[:])

            # A natural, packed [128, 16*KC]: chunk c at cols 16c = A[128c:128c+128,:]
            a_nat = const.tile([128, RANK * KC], F32)
            nc.sync.dma_start(
                a_nat[:].rearrange("p (c r) -> p c r", r=RANK),
                a[:].rearrange("(c p) r -> p c r", p=128),
            )
            # AT [16, D] = A.T
            at = const.tile([RANK, D], F32)
            for c in range(KC):
                ps = pre_ps.tile([RANK, 128], F32)
                nc.tensor.transpose(
                    ps[:], a_nat[:, RANK * c:RANK * (c + 1)], ident[:]
                )
                nc.scalar.copy(at[:, 128 * c:128 * (c + 1)], ps[:])

            # ---- main loop over row tiles ----
            for m in range(N_RT):
                x_t = xin.tile([RT, D], F32)
                nc.sync.dma_start(x_t[:], x[RT * m:RT * (m + 1), :])

                tt_ps = tt_ps_pool.tile([RANK, RT], F32)
                for g in range(KC // TG):
                    xt_ps = xt_ps_pool.tile([128, 128 * TG], F32)
                    for s in range(TG):
                        c = g * TG + s
                        nc.tensor.transpose(
                            xt_ps[:, 128 * s:128 * (s + 1)],
                            x_t[:, 128 * c:128 * (c + 1)],
                            ident[:],
                        )
                    xt_sb = xt_sb_pool.tile([128, 128 * TG], F32)
                    nc.scalar.copy(xt_sb[:], xt_ps[:])
                    for s in range(TG):
                        c = g * TG + s
                        nc.tensor.matmul(
                            tt_ps[:],
                            bt[:, RANK * c:RANK * (c + 1)],
                            xt_sb[:, 128 * s:128 * (s + 1)],
                            start=(c == 0),
                            stop=(c == KC - 1),
                        )

                tt_sb = tt_sb_pool.tile([RANK, RT], F32)
                nc.vector.tensor_copy(tt_sb[:], tt_ps[:])

                o_sb = o_sb_pool.tile([RT, D], F32)
                for j in range(NJ):
                    o_ps = o_ps_pool.tile([RT, 512], F32)
                    nc.tensor.matmul(
                        o_ps[:], tt_sb[:], at[:, 512 * j:512 * (j + 1)],
                        start=True, stop=True,
                    )
                    nc.vector.tensor_copy(o_sb[:, 512 * j:512 * (j + 1)], o_ps[:])
                nc.sync.dma_start(y[RT * m:RT * (m + 1), :], o_sb[:])

    nc.compile()
    return nc


def get_nc():
    if "nc" not in _CACHE:
        _CACHE["nc"] = _build()
    return _CACHE["nc"]


def make_in_maps(input, A, B):
    xf = np.ascontiguousarray(np.asarray(input, dtype=np.float32).reshape(ROWS_TOTAL, D))
    A = np.ascontiguousarray(np.asarray(A, dtype=np.float32))
    B = np.ascontiguousarray(np.asarray(B, dtype=np.float32))
    return [
        {"x": xf[R * i:R * (i + 1)], "a": A, "b": B}
        for i in range(N_CORES)
    ]


def kernel(input, A, B):
    nc = get_nc()
    in_maps = make_in_maps(input, A, B)
    res = run_bass_kernel_spmd(nc, in_maps, core_ids=list(range(N_CORES)))
    out = np.concatenate([r["y"] for r in res.results], axis=0)
    return out.reshape(input.shape)
